# revision 1
# baseline (speedup 1.0000x reference)
"""Multi-head attention (B=2, S=2048, D=768, H=12) on 8 Trainium2 NeuronCores.

Sharding: core c handles batch b=c//4 and heads 3*(c%4) .. 3*(c%4)+2.
Each core:
  1. Projects Q,K (feature-major, transposed) and V (sequence-major, with an
     appended ones-column for the softmax denominator) for its 3 heads.
  2. Computes scores^T = K @ Q^T per head (contraction over head_dim=64, heads
     paired into PE row-groups), exp on ScalarE (scores are O(1), no max
     subtraction needed), then ctx^T_aug = V_aug^T @ exp(scores^T) which yields
     both the unnormalized context and the softmax denominator in one pass.
  3. Normalizes, writes local ctx^T [192, 2048] to DRAM.
  4. One 8-rank AllGather -> ctx^T for all heads/batches [1536, 2048].
  5. Indirect-gathers its (batch, s_q quarter) slice and computes the output
     projection y^T[:, q*512:(q+1)*512] = Wo^T @ ctx^T + bo.
Host assembles y[b, q*512:(q+1)*512, :] = out_c^T.

All matmul operands are float32r (TF32-like, full PE rate); accumulation fp32.
"""
import sys

if "/opt/trn_rl_repo" not in sys.path:
    sys.path.insert(0, "/opt/trn_rl_repo")

import numpy as np

B, S, D, H = 2, 2048, 768, 12
HD = 64
P = 128
N_CORES = 8
HPC = 3          # heads per core
NQ = 4           # s_q chunks of 512
SK = 16          # s_k chunks of 128
KD = 6           # D chunks of 128
W = 512          # working free-dim chunk

_CACHE = {}


def _install_profile_shim():
    """run_bass_kernel_spmd(trace=True) needs antenv.axon_hooks; provide it."""
    import contextlib
    import ctypes
    import types

    if "antenv.axon_hooks" in sys.modules:
        return
    try:
        lib = ctypes.CDLL("/opt/axon/libaxon_pjrt.so")
    except OSError:
        return
    if not hasattr(lib, "axon_start_nrt_profile"):
        return
    lib.axon_start_nrt_profile.argtypes = [
        ctypes.POINTER(ctypes.c_int64),
        ctypes.c_size_t,
    ]
    lib.axon_start_nrt_profile.restype = ctypes.c_int64
    lib.axon_stop_nrt_profile.argtypes = [ctypes.c_char_p]
    lib.axon_stop_nrt_profile.restype = ctypes.c_int64

    @contextlib.contextmanager
    def _hook(output_dir, device_ids):
        import jax

        jax.devices()
        if device_ids:
            ids = (ctypes.c_int64 * len(device_ids))(*device_ids)
            rc = lib.axon_start_nrt_profile(ids, len(device_ids))
        else:
            rc = lib.axon_start_nrt_profile(None, 0)
        if rc != 0:
            raise RuntimeError(f"axon_start_nrt_profile rc={rc}")
        try:
            yield
        finally:
            n = lib.axon_stop_nrt_profile(str(output_dir).encode())
            if n < 0:
                raise RuntimeError(f"axon_stop_nrt_profile rc={n}")

    mod = types.ModuleType("antenv.axon_hooks")
    mod.get_axon_ntff_profile_hook = lambda: _hook
    mod.set_axon_ntff_profile_hook = lambda h: None
    sys.modules["antenv.axon_hooks"] = mod


def _build():
    import concourse.bass as bass
    from concourse import bacc
    import concourse.tile as tile
    import concourse.mybir as mybir

    f32r = mybir.dt.float32r
    f32 = mybir.dt.float32
    u32 = mybir.dt.uint32
    AF = mybir.ActivationFunctionType
    ALU = mybir.AluOpType

    nc = bacc.Bacc("TRN2", target_bir_lowering=False, debug=False,
                   num_devices=N_CORES)

    xT = nc.dram_tensor("xT", [D, S], f32r, kind="ExternalInput")
    w_qk = nc.dram_tensor("w_qk", [D, 384], f32r, kind="ExternalInput")
    b_qk = nc.dram_tensor("b_qk", [384, 1], f32, kind="ExternalInput")
    w_v = nc.dram_tensor("w_v", [D, 256], f32r, kind="ExternalInput")
    b_v = nc.dram_tensor("b_v", [1, 256], f32, kind="ExternalInput")
    w_o = nc.dram_tensor("w_o", [D, D], f32r, kind="ExternalInput")
    b_o = nc.dram_tensor("b_o", [D, 1], f32, kind="ExternalInput")
    gidx = nc.dram_tensor("gidx", [D, 1], u32, kind="ExternalInput")
    zin = nc.dram_tensor("zin", [P, P], f32r, kind="ExternalInput")
    out = nc.dram_tensor("out", [D, W], f32r, kind="ExternalOutput")

    cc_in = nc.dram_tensor("cc_in", [NQ, HPC * HD, W], f32r)
    cc_all = nc.dram_tensor("cc_all", [NQ * N_CORES * HPC * HD, W], f32r,
                            addr_space="Shared")

    with tile.TileContext(nc) as tc:
        with tc.tile_pool(name="const", bufs=1) as const, \
             tc.tile_pool(name="qkp", bufs=1) as qkp, \
             tc.tile_pool(name="vp", bufs=1) as vp, \
             tc.tile_pool(name="work", bufs=4) as work, \
             tc.tile_pool(name="expp", bufs=4) as expp, \
             tc.tile_pool(name="gat", bufs=1) as gat, \
             tc.tile_pool(name="outp", bufs=3) as outp:

            # ---- constant loads -------------------------------------------
            zeros_t = const.tile([P, P], f32r, tag="zeros")
            nc.sync.dma_start(out=zeros_t, in_=zin[:, :])
            wqk = []
            xt = []
            for k in range(KD):
                t = const.tile([P, 384], f32r, tag=f"wqk{k}")
                nc.sync.dma_start(out=t, in_=w_qk[k * P:(k + 1) * P, :])
                wqk.append(t)
            for k in range(KD):
                t = const.tile([P, S], f32r, tag=f"xt{k}", name=f"xt{k}")
                xt.append(t)
            for k in range(KD):
                nc.scalar.dma_start(out=xt[k][:, 0:1024],
                                    in_=xT[k * P:(k + 1) * P, 0:1024])
            for k in range(KD):
                nc.scalar.dma_start(out=xt[k][:, 1024:2048],
                                    in_=xT[k * P:(k + 1) * P, 1024:2048])
            wv = []
            for k in range(KD):
                t = const.tile([P, 256], f32r, tag=f"wv{k}")
                nc.sync.dma_start(out=t, in_=w_v[k * P:(k + 1) * P, :])
                wv.append(t)
            bqk = []
            for m in range(3):
                t = const.tile([P, 1], f32, tag=f"bqk{m}")
                nc.sync.dma_start(out=t, in_=b_qk[m * P:(m + 1) * P, :])
                bqk.append(t)
            bv = const.tile([P, 256], f32, tag="bv")
            bv_bcast = bass.AP(tensor=b_v[:, :].tensor, offset=0,
                               ap=[[0, P], [1, 256]])
            nc.gpsimd.dma_start(out=bv, in_=bv_bcast)
            wo = []
            bo = []
            gix = []
            for k in range(KD):
                t = const.tile([P, D], f32r, tag=f"wo{k}")
                nc.sync.dma_start(out=t, in_=w_o[k * P:(k + 1) * P, :])
                wo.append(t)
                t = const.tile([P, 1], f32, tag=f"bo{k}")
                nc.sync.dma_start(out=t, in_=b_o[k * P:(k + 1) * P, :])
                bo.append(t)
                t = const.tile([P, 1], u32, tag=f"gix{k}")
                nc.sync.dma_start(out=t, in_=gidx[k * P:(k + 1) * P, :])
                gix.append(t)

            # ---- attention -----------------------------------------------
            # Chunk = one [s_k 128, s_q 512] score block for one head.
            # Groups of 2 chunks share a 2-bank PSUM tile so one ACT exp
            # covers 1024 columns (amortizes the ~352-cycle ACT overhead).
            # Software-pipelined emission: mm_s(g+1) is emitted before
            # mm_c(g) so the PE never stalls behind the ACT.
            qkt = [qkp.tile([P, S], f32r, tag=f"qkt{m}", name=f"qkt{m}")
                   for m in range(3)]
            q2c = qkp.tile([64, S], f32r, tag="q2c")
            vsb = [vp.tile([P, 256], f32r, tag=f"v{s}", name=f"v{s}")
                   for s in range(SK)]

            def normalize(pc, nq, h):
                rec = work.tile([1, W], f32, tag="rec")
                nc.vector.reciprocal(rec[0:1, :], pc[64:65, :])
                rb = work.tile([64, W], f32, tag="rb")
                nc.gpsimd.partition_broadcast(rb, rec[:1, :])
                ctx = work.tile([64, W], f32r, tag="ctx")
                nc.vector.tensor_tensor(out=ctx, in0=pc[0:64, :], in1=rb,
                                        op=ALU.mult)
                nc.gpsimd.dma_start(
                    out=cc_in[nq, h * HD:(h + 1) * HD, :],
                    in_=ctx)
                norm_done.setdefault(nq, set()).add(h)
                if norm_done[nq] == {0, 1, 2}:
                    nc.gpsimd.collective_compute(
                        "AllGather", ALU.bypass,
                        ins=[cc_in[nq]],
                        outs=[cc_all[nq * 1536:(nq + 1) * 1536, :]],
                        replica_groups=[list(range(N_CORES))])

            # build group list: per nq, pair phase then solo phase
            groups = []
            for nq in range(NQ):
                for sk in range(SK):
                    groups.append({"nq": nq, "chunks": [(0, sk), (1, sk)],
                                   "last": False})
                for sk in range(0, SK, 2):
                    g = {"nq": nq, "chunks": [(2, sk), (2, sk + 1)],
                         "last": sk == SK - 2}
                    groups.append(g)

            pc_tiles = {}
            cnt = {}
            norm_done = {}
            ag_fired = set()

            def emit_mm_s(gi, grp):
                nq = grp["nq"]
                eps = psE.tile([P, 2 * W], f32, tag="ea" if gi % 2 == 0
                               else "eb", name=f"eps{gi}")
                for j, (h, sk) in enumerate(grp["chunks"]):
                    if h == 0:
                        lhsT = qkt[0][0:64, sk * P:(sk + 1) * P]
                        rhs = qkt[1][0:64, nq * W:(nq + 1) * W]
                        tp = (0, 0)
                    elif h == 1:
                        lhsT = qkt[0][64:128, sk * P:(sk + 1) * P]
                        rhs = qkt[1][64:128, nq * W:(nq + 1) * W]
                        tp = (64, 0)
                    else:
                        lhsT = qkt[2][0:64, sk * P:(sk + 1) * P]
                        rhs = q2c[:, nq * W:(nq + 1) * W]
                        tp = (0, 0)
                    nc.tensor.matmul(eps[:, j * W:(j + 1) * W], lhsT, rhs,
                                     start=True, stop=True, tile_position=tp)
                esb = expp.tile([P, 2 * W], f32r, tag="e", name=f"esb{gi}")
                nc.scalar.activation(esb, eps, AF.Exp)
                return esb

            def emit_mm_c(grp, esb):
                nq = grp["nq"]
                for j, (h, sk) in enumerate(grp["chunks"]):
                    key = (nq, h)
                    if key not in pc_tiles:
                        pc_tiles[key] = psC.tile([65, W], f32, tag="pc",
                                                 name=f"pc{nq}_{h}")
                        cnt[key] = 0
                    nc.tensor.matmul(
                        pc_tiles[key],
                        vsb[sk][:, h * 65:h * 65 + 65],
                        esb[:, j * W:(j + 1) * W],
                        start=(cnt[key] == 0), stop=(cnt[key] == SK - 1))
                    cnt[key] += 1
                    if cnt[key] == SK:
                        normalize(pc_tiles[key], nq, h)

            with tc.tile_pool(name="ps_proj", bufs=4, space="PSUM") as psP:

                def emit_qk_block(n):
                    for m in range(3):
                        ps = psP.tile([P, W], f32, tag="proj",
                                      name=f"psqk{n}_{m}")
                        first = n == 0 and m == 0
                        if first:
                            # zero-contribution warmup: keeps the PE busy
                            # while x DMAs land so HAM reaches 2.4GHz; the
                            # two regions cover [0:512] so has_written is
                            # clean for the real accumulation below
                            for d in range(24):
                                if d % 2 == 0:
                                    nc.tensor.matmul(
                                        ps[:, 0:384], zeros_t, wqk[0][:, :],
                                        start=(d == 0), stop=False,
                                        skip_group_check=True)
                                else:
                                    nc.tensor.matmul(
                                        ps[:, 384:512], zeros_t,
                                        wqk[1][:, 0:128],
                                        start=(d == 1), stop=False,
                                        skip_group_check=True)
                        for k in range(KD):
                            nc.tensor.matmul(
                                ps,
                                wqk[k][:, m * P:(m + 1) * P],
                                xt[k][:, n * W:(n + 1) * W],
                                start=(k == 0 and not first),
                                stop=(k == KD - 1),
                                skip_group_check=first)
                        nc.vector.tensor_scalar_add(
                            qkt[m][:, n * W:(n + 1) * W], ps, bqk[m])
                    nc.sync.dma_start(out=q2c[:, n * W:(n + 1) * W],
                                      in_=qkt[2][64:128, n * W:(n + 1) * W])

                for n in range(NQ):
                    emit_qk_block(n)

            with tc.tile_pool(name="ps_v", bufs=1, space="PSUM") as psV, \
                 tc.tile_pool(name="ps_e", bufs=1, space="PSUM") as psE, \
                 tc.tile_pool(name="ps_c", bufs=3, space="PSUM") as psC:

                def emit_v_block(n):
                    for s_ in range(4 * n, 4 * n + 4):
                        ps = psV.tile([P, W], f32, tag="projv",
                                      name=f"psv{s_}")
                        for k in range(KD):
                            nc.tensor.matmul(
                                ps[:, 0:256],
                                xt[k][:, s_ * P:(s_ + 1) * P],
                                wv[k],
                                start=(k == 0), stop=(k == KD - 1))
                        nc.vector.tensor_tensor(out=vsb[s_], in0=ps[:, 0:256],
                                                in1=bv, op=ALU.add)

                # head start: emit the first 3 attention groups' score
                # matmuls + exps BEFORE the V projection so the ACT stream
                # begins while the PE grinds through V. Their context
                # matmuls are deferred until V lands (depth-3 pipeline).
                HS = 3
                pending = []
                for gi, grp in enumerate(groups):
                    if gi == HS:
                        for n in range(NQ):
                            emit_v_block(n)
                    esb = emit_mm_s(gi, grp)
                    pending.append((grp, esb))
                    if gi >= HS:
                        emit_mm_c(*pending.pop(0))
                while pending:
                    emit_mm_c(*pending.pop(0))

            # ---- gather + output projection ------------------------------
            ctxg = []
            for k in range(KD):
                t = gat.tile([P, W], f32r, tag=f"ctxg{k}", name=f"ctxg{k}")
                nc.gpsimd.indirect_dma_start(
                    out=t,
                    out_offset=None,
                    in_=cc_all[:, :],
                    in_offset=bass.IndirectOffsetOnAxis(ap=gix[k][:, :1],
                                                        axis=0),
                )
                ctxg.append(t)
            with tc.tile_pool(name="ps_y", bufs=2, space="PSUM") as py:
                for m in range(KD):
                    ps = py.tile([P, W], f32)
                    if m == 0:
                        # warm the PE during the AllGather wait so the
                        # output projection runs at full clock
                        for d in range(16):
                            nc.tensor.matmul(
                                ps, zeros_t, wo[0][:, 0:W],
                                start=(d == 0), stop=False,
                                skip_group_check=True)
                    for k in range(KD):
                        nc.tensor.matmul(
                            ps,
                            wo[k][:, m * P:(m + 1) * P],
                            ctxg[k],
                            start=(k == 0 and m != 0),
                            stop=(k == KD - 1),
                            skip_group_check=(m == 0))
                    yt = outp.tile([P, W], f32r, tag="yt")
                    nc.vector.tensor_scalar_add(yt, ps, bo[m])
                    nc.gpsimd.dma_start(out=out[m * P:(m + 1) * P, :], in_=yt)

    nc.compile()
    return nc


def _get_nc():
    if "nc" not in _CACHE:
        _install_profile_shim()
        _CACHE["nc"] = _build()
    return _CACHE["nc"]


def _make_in_maps(x, Wq, bq, Wk, bk, Wv, bv, Wo, bo):
    scale = np.float32(1.0 / np.sqrt(HD))
    f = np.float32
    x, Wq, bq, Wk, bk, Wv, bv, Wo, bo = [
        np.asarray(a, dtype=f) for a in (x, Wq, bq, Wk, bk, Wv, bv, Wo, bo)]

    in_maps = []
    for c in range(N_CORES):
        b = c // 4
        hs = (c % 4) * HPC
        q = c % 4
        hh = [hs, hs + 1, hs + 2]

        def wc(Wm, h):
            return Wm[:, h * HD:(h + 1) * HD]

        def bc(bm, h):
            return bm[h * HD:(h + 1) * HD]

        xTb = np.ascontiguousarray(x[b].T)
        w_qk = np.concatenate(
            [wc(Wk, hh[0]), wc(Wk, hh[1]),
             wc(Wq, hh[0]) * scale, wc(Wq, hh[1]) * scale,
             wc(Wk, hh[2]), wc(Wq, hh[2]) * scale], axis=1)
        b_qk = np.concatenate(
            [bc(bk, hh[0]), bc(bk, hh[1]),
             bc(bq, hh[0]) * scale, bc(bq, hh[1]) * scale,
             bc(bk, hh[2]), bc(bq, hh[2]) * scale])[:, None]
        w_v = np.zeros((D, 256), dtype=f)
        b_v = np.zeros((1, 256), dtype=f)
        for i, h in enumerate(hh):
            w_v[:, i * 65:i * 65 + HD] = wc(Wv, h)
            b_v[0, i * 65:i * 65 + HD] = bc(bv, h)
            b_v[0, i * 65 + HD] = 1.0
        i_feat = np.arange(D, dtype=np.uint32)
        g = q * 1536 + (4 * b + i_feat // 192) * 192 + (i_feat % 192)
        in_maps.append({
            "xT": np.ascontiguousarray(xTb),
            "w_qk": np.ascontiguousarray(w_qk),
            "b_qk": np.ascontiguousarray(b_qk),
            "w_v": w_v,
            "b_v": b_v,
            "w_o": np.ascontiguousarray(Wo),
            "b_o": np.ascontiguousarray(bo[:, None]),
            "gidx": g.astype(np.uint32)[:, None],
            "zin": np.zeros((P, P), dtype=f),
        })
    return in_maps


def kernel(x, Wq, bq, Wk, bk, Wv, bv, Wo, bo, _trace=False):
    from concourse.bass_utils import run_bass_kernel_spmd

    nc = _get_nc()
    in_maps = _make_in_maps(x, Wq, bq, Wk, bk, Wv, bv, Wo, bo)
    res = run_bass_kernel_spmd(nc, in_maps, list(range(N_CORES)),
                               trace=_trace)
    _CACHE["last_results"] = res
    y = np.empty((B, S, D), dtype=np.float32)
    for c in range(N_CORES):
        b = c // 4
        q = c % 4
        y[b, q * W:(q + 1) * W, :] = res.results[c]["out"].T
    return y



# revision 13
# speedup vs baseline: 1.1045x; 1.1045x over previous
"""Multi-head attention (B=2, S=2048, D=768, H=12) on 8 Trainium2 NeuronCores.

Sharding: core c handles batch b=c//4 and heads 3*(c%4) .. 3*(c%4)+2.

v2: ACT(exp)-centric schedule. The softmax exp is 12.58M elements/core on the
Scalar engine (~0.83ns/col + ~190ns/instr) ~= 100us — the hard floor. The
kernel is organized as one long ACT-saturated attention phase:

  1. All matmul operands are bf16 (fp32 PSUM accumulation): halves x DMA,
     SBUF footprint, and collective bytes. PE rate is unchanged (1 cyc/row
     for both fp32r and bf16).
  2. QK/V projections are split into small pieces and emitted between
     attention groups so the exp stream starts at ~8us; only QK block 0 and
     V block 0 are emitted up front.
  3. Per (s_q 512-quarter): scores^T = K @ Q^T per head (heads paired into PE
     row-halves via tile_position), exp on ScalarE ([128,1024] per
     instruction), ctx^T_aug = V_aug^T @ exp(scores^T) accumulated per head
     in one PSUM bank (ones-column gives the softmax denominator).
  4. Normalize: reciprocal_approx_fast (single DVE op) on the denominator
     row, gpsimd partition_broadcast, one DVE multiply -> bf16 ctx.
  5. Per quarter, one 4-rank batch-group AllGather (bf16) delivers
     ctx^T[768, 512] in head order; core q=c%4 indirect-gathers quarter q and
     computes y^T[:, q*512:(q+1)*512] = Wo^T @ ctx^T + bo with the PE kept
     warm through the collective wait.

Host assembles y[b, q*512:(q+1)*512, :] = out_c^T.
"""
import sys

if "/opt/trn_rl_repo" not in sys.path:
    sys.path.insert(0, "/opt/trn_rl_repo")

import numpy as np

B, S, D, H = 2, 2048, 768, 12
HD = 64
P = 128
N_CORES = 8
HPC = 3          # heads per core
NQ = 4           # s_q chunks of 512
SK = 16          # s_k chunks of 128
KD = 6           # D chunks of 128
W = 512          # working free-dim chunk
VW = 3 * 65      # packed V_aug width (3 heads x (64 + ones column))

_CACHE = {}


def _install_profile_shim():
    """run_bass_kernel_spmd(trace=True) needs antenv.axon_hooks; provide it."""
    import contextlib
    import ctypes
    import types

    if "antenv.axon_hooks" in sys.modules:
        return
    try:
        lib = ctypes.CDLL("/opt/axon/libaxon_pjrt.so")
    except OSError:
        return
    if not hasattr(lib, "axon_start_nrt_profile"):
        return
    lib.axon_start_nrt_profile.argtypes = [
        ctypes.POINTER(ctypes.c_int64),
        ctypes.c_size_t,
    ]
    lib.axon_start_nrt_profile.restype = ctypes.c_int64
    lib.axon_stop_nrt_profile.argtypes = [ctypes.c_char_p]
    lib.axon_stop_nrt_profile.restype = ctypes.c_int64

    @contextlib.contextmanager
    def _hook(output_dir, device_ids):
        import jax

        jax.devices()
        if device_ids:
            ids = (ctypes.c_int64 * len(device_ids))(*device_ids)
            rc = lib.axon_start_nrt_profile(ids, len(device_ids))
        else:
            rc = lib.axon_start_nrt_profile(None, 0)
        if rc != 0:
            raise RuntimeError(f"axon_start_nrt_profile rc={rc}")
        try:
            yield
        finally:
            n = lib.axon_stop_nrt_profile(str(output_dir).encode())
            if n < 0:
                raise RuntimeError(f"axon_stop_nrt_profile rc={n}")

    mod = types.ModuleType("antenv.axon_hooks")
    mod.get_axon_ntff_profile_hook = lambda: _hook
    mod.set_axon_ntff_profile_hook = lambda h: None
    sys.modules["antenv.axon_hooks"] = mod


import os

DEBUG_DUMP = bool(os.environ.get("KERNEL_DEBUG_DUMP"))


def _build():
    import concourse.bass as bass
    from concourse import bacc
    import concourse.tile as tile
    import concourse.mybir as mybir

    bf16 = mybir.dt.bfloat16
    f32 = mybir.dt.float32
    u32 = mybir.dt.uint32
    AF = mybir.ActivationFunctionType
    ALU = mybir.AluOpType

    nc = bacc.Bacc("TRN2", target_bir_lowering=False, debug=False,
                   num_devices=N_CORES)

    xT = nc.dram_tensor("xT", [D, S], bf16, kind="ExternalInput")
    w_qk = nc.dram_tensor("w_qk", [D, 384], bf16, kind="ExternalInput")
    b_qk = nc.dram_tensor("b_qk", [384, 1], f32, kind="ExternalInput")
    w_v = nc.dram_tensor("w_v", [D, VW], bf16, kind="ExternalInput")
    b_v = nc.dram_tensor("b_v", [1, VW], f32, kind="ExternalInput")
    w_o = nc.dram_tensor("w_o", [D, D], bf16, kind="ExternalInput")
    b_o = nc.dram_tensor("b_o", [D, 1], f32, kind="ExternalInput")
    gidx = nc.dram_tensor("gidx", [D, 1], u32, kind="ExternalInput")
    zin = nc.dram_tensor("zin", [P, P], bf16, kind="ExternalInput")
    out = nc.dram_tensor("out", [D, W], f32, kind="ExternalOutput")

    cc_in = nc.dram_tensor("cc_in", [NQ, HPC * HD, W], bf16)
    cc_all = nc.dram_tensor("cc_all", [NQ * 4 * HPC * HD, W], bf16)
    if DEBUG_DUMP:
        dbg_qkt = nc.dram_tensor("dbg_qkt", [3 * P, S], bf16,
                                 kind="ExternalOutput")
        dbg_v = nc.dram_tensor("dbg_v", [P, VW], bf16, kind="ExternalOutput")
        dbg_esb = nc.dram_tensor("dbg_esb", [P, 2 * W], bf16,
                                 kind="ExternalOutput")
        dbg_ccin = nc.dram_tensor("dbg_ccin", [NQ * HPC * HD, W], bf16,
                                  kind="ExternalOutput")
        dbg_ccall = nc.dram_tensor("dbg_ccall", [NQ * 4 * HPC * HD, W], bf16,
                                   kind="ExternalOutput")
        dbg_ctxg = nc.dram_tensor("dbg_ctxg", [D, W], bf16,
                                  kind="ExternalOutput")
        dbg_den = nc.dram_tensor("dbg_den", [2, W], f32,
                                 kind="ExternalOutput")
    GROUPS = [[0, 1, 2, 3], [4, 5, 6, 7]]

    with tile.TileContext(nc) as tc:
        with tc.tile_pool(name="const", bufs=1) as const, \
             tc.tile_pool(name="qkp", bufs=1) as qkp, \
             tc.tile_pool(name="vp", bufs=1) as vp, \
             tc.tile_pool(name="work", bufs=4) as work, \
             tc.tile_pool(name="expp", bufs=4) as expp, \
             tc.tile_pool(name="gat", bufs=1) as gat, \
             tc.tile_pool(name="outp", bufs=3) as outp:

            # ---- constant / input loads -----------------------------------
            # sync queue: zeros then all of x (no deps -> streams in ASAP),
            # then wo/gidx/bo which are only needed at the tail.
            zeros_t = const.tile([P, P], bf16, tag="zeros")
            nc.sync.dma_start(out=zeros_t, in_=zin[:, :])
            xt = [const.tile([P, S], bf16, tag=f"xt{k}", name=f"xt{k}")
                  for k in range(KD)]
            for n in range(NQ):
                for k in range(KD):
                    nc.sync.dma_start(
                        out=xt[k][:, n * W:(n + 1) * W],
                        in_=xT[k * P:(k + 1) * P, n * W:(n + 1) * W])

            # gpsimd queue: projection weights + biases.
            wqk = []
            for k in range(KD):
                t = const.tile([P, 384], bf16, tag=f"wqk{k}")
                nc.gpsimd.dma_start(out=t, in_=w_qk[k * P:(k + 1) * P, :])
                wqk.append(t)
            bqk = []
            for m in range(3):
                t = const.tile([P, 1], f32, tag=f"bqk{m}")
                nc.gpsimd.dma_start(out=t, in_=b_qk[m * P:(m + 1) * P, :])
                bqk.append(t)
            bv = const.tile([P, VW], f32, tag="bv")
            bv_bcast = bass.AP(tensor=b_v[:, :].tensor, offset=0,
                               ap=[[0, P], [1, VW]])
            nc.gpsimd.dma_start(out=bv, in_=bv_bcast)
            wv = []
            for k in range(KD):
                t = const.tile([P, VW], bf16, tag=f"wv{k}")
                nc.gpsimd.dma_start(out=t, in_=w_v[k * P:(k + 1) * P, :])
                wv.append(t)

            # tail constants on sync (queued behind x; overlap attention)
            wo = []
            bo = []
            gix = []
            for k in range(KD):
                t = const.tile([P, D], bf16, tag=f"wo{k}")
                nc.sync.dma_start(out=t, in_=w_o[k * P:(k + 1) * P, :])
                wo.append(t)
                t = const.tile([P, 1], f32, tag=f"bo{k}")
                nc.sync.dma_start(out=t, in_=b_o[k * P:(k + 1) * P, :])
                bo.append(t)
                t = const.tile([P, 1], u32, tag=f"gix{k}")
                nc.sync.dma_start(out=t, in_=gidx[k * P:(k + 1) * P, :])
                gix.append(t)

            # ---- attention state ------------------------------------------
            qkt = [qkp.tile([P, S], bf16, tag=f"qkt{m}", name=f"qkt{m}")
                   for m in range(3)]
            q2c = qkp.tile([64, S], bf16, tag="q2c")
            vsb = [vp.tile([P, VW], bf16, tag=f"v{s}", name=f"v{s}")
                   for s in range(SK)]

            with tc.tile_pool(name="ps_proj", bufs=2, space="PSUM") as psP, \
                 tc.tile_pool(name="ps_e", bufs=1, space="PSUM") as psE, \
                 tc.tile_pool(name="ps_c", bufs=2, space="PSUM") as psC:

                # ---- projection pieces ------------------------------------
                def emit_qk_piece(n, m, warm=False):
                    ps = psP.tile([P, W], f32, tag="proj",
                                  name=f"psqk{n}_{m}")
                    if warm:
                        # zero-contribution warmup: ramps the PE clock while
                        # the x DMAs land; covers [0:512] so has_written is
                        # clean for the real accumulation below
                        for d in range(16):
                            if d % 2 == 0:
                                nc.tensor.matmul(
                                    ps[:, 0:384], zeros_t, wqk[0][:, :],
                                    start=(d == 0), stop=False,
                                    skip_group_check=True)
                            else:
                                nc.tensor.matmul(
                                    ps[:, 384:512], zeros_t,
                                    wqk[1][:, 0:128],
                                    start=(d == 1), stop=False,
                                    skip_group_check=True)
                    for k in range(KD):
                        nc.tensor.matmul(
                            ps,
                            wqk[k][:, m * P:(m + 1) * P],
                            xt[k][:, n * W:(n + 1) * W],
                            start=(k == 0 and not warm),
                            stop=(k == KD - 1),
                            skip_group_check=warm)
                    nc.vector.tensor_scalar_add(
                        qkt[m][:, n * W:(n + 1) * W], ps, bqk[m])
                    if m == 2:
                        nc.gpsimd.dma_start(
                            out=q2c[:, n * W:(n + 1) * W],
                            in_=qkt[2][64:128, n * W:(n + 1) * W])

                def emit_v_piece(s0, cnt=2):
                    for s_ in range(s0, s0 + cnt):
                        ps = psP.tile([P, W], f32, tag="proj",
                                      name=f"psv{s_}")
                        for k in range(KD):
                            nc.tensor.matmul(
                                ps[:, 0:VW],
                                xt[k][:, s_ * P:(s_ + 1) * P],
                                wv[k],
                                start=(k == 0), stop=(k == KD - 1))
                        nc.vector.tensor_tensor(out=vsb[s_], in0=ps[:, 0:VW],
                                                in1=bv, op=ALU.add)

                # ---- attention groups -------------------------------------
                # Chunk = one [s_k 128, s_q 512] score block for one head.
                # Groups of 2 chunks share a 2-bank PSUM tile so one ACT exp
                # covers 1024 columns. Software-pipelined emission with ctx
                # lagging scores by 3 groups.
                groups = []
                for nq in range(NQ):
                    for sk in range(SK):
                        groups.append({"nq": nq, "chunks": [(0, sk), (1, sk)]})
                    for sk in range(0, SK, 2):
                        groups.append({"nq": nq,
                                       "chunks": [(2, sk), (2, sk + 1)]})

                # projection pieces interleaved into the nq=0 group stream:
                # gi -> list of thunks. K rows for pairs sk=4n need qk(n,m=0)
                # before gi=4n; vsb[sk] needed by ctx(sk) (lag 3); K2/Q2
                # (m=2) needed by the solo phase at gi=16; Q blocks (m=1)
                # needed by nq>=1 pairs at gi=24.
                pieces = {
                    1: [lambda: emit_qk_piece(1, 0)],
                    2: [lambda: emit_v_piece(4)],
                    3: [lambda: emit_v_piece(6)],
                    5: [lambda: emit_qk_piece(2, 0)],
                    6: [lambda: emit_v_piece(8)],
                    7: [lambda: emit_v_piece(10)],
                    9: [lambda: emit_qk_piece(3, 0)],
                    10: [lambda: emit_v_piece(12)],
                    11: [lambda: emit_v_piece(14)],
                    12: [lambda: emit_qk_piece(1, 2)],
                    13: [lambda: emit_qk_piece(2, 2)],
                    14: [lambda: emit_qk_piece(3, 2)],
                    17: [lambda: emit_qk_piece(1, 1)],
                    19: [lambda: emit_qk_piece(2, 1)],
                    21: [lambda: emit_qk_piece(3, 1)],
                }

                pc_tiles = {}
                cnt = {}
                norm_done = {}

                def normalize(pc, nq, h):
                    # custom-DVE ops drop the input partition base offset, so
                    # stage the denominator row at partition 0 first
                    den = work.tile([1, W], f32, tag="den")
                    nc.vector.tensor_scalar_mul(den, pc[64:65, :], 1.0)
                    rec = work.tile([1, W], f32, tag="rec")
                    nc.vector.reciprocal_approx_fast(rec[0:1, :],
                                                     den[0:1, :])
                    if DEBUG_DUMP and nq == 0 and h == 0:
                        nc.sync.dma_start(out=dbg_den[0:1, :],
                                          in_=den[0:1, :])
                        nc.sync.dma_start(out=dbg_den[1:2, :],
                                          in_=rec[0:1, :])
                    rb = work.tile([64, W], f32, tag="rb")
                    nc.gpsimd.partition_broadcast(rb, rec[:1, :])
                    ctx = work.tile([64, W], bf16, tag="ctx")
                    nc.vector.tensor_tensor(out=ctx, in0=pc[0:64, :], in1=rb,
                                            op=ALU.mult)
                    nc.gpsimd.dma_start(
                        out=cc_in[nq, h * HD:(h + 1) * HD, :],
                        in_=ctx)
                    norm_done.setdefault(nq, set()).add(h)
                    if norm_done[nq] == {0, 1, 2}:
                        nc.gpsimd.collective_compute(
                            "AllGather", ALU.bypass,
                            ins=[cc_in[nq]],
                            outs=[cc_all[nq * 768:(nq + 1) * 768, :]],
                            replica_groups=GROUPS)

                def emit_mm_s(gi, grp):
                    nq = grp["nq"]
                    eps = psE.tile([P, 2 * W], f32, tag="ea" if gi % 2 == 0
                                   else "eb", name=f"eps{gi}")
                    for j, (h, sk) in enumerate(grp["chunks"]):
                        if h == 0:
                            lhsT = qkt[0][0:64, sk * P:(sk + 1) * P]
                            rhs = qkt[1][0:64, nq * W:(nq + 1) * W]
                            tp = (0, 0)
                        elif h == 1:
                            lhsT = qkt[0][64:128, sk * P:(sk + 1) * P]
                            rhs = qkt[1][64:128, nq * W:(nq + 1) * W]
                            tp = (64, 0)
                        else:
                            lhsT = qkt[2][0:64, sk * P:(sk + 1) * P]
                            rhs = q2c[:, nq * W:(nq + 1) * W]
                            tp = (0, 0)
                        nc.tensor.matmul(eps[:, j * W:(j + 1) * W], lhsT, rhs,
                                         start=True, stop=True,
                                         tile_position=tp)
                    esb = expp.tile([P, 2 * W], bf16, tag="e",
                                    name=f"esb{gi}")
                    nc.scalar.activation(esb, eps, AF.Exp)
                    if DEBUG_DUMP and gi == 0:
                        nc.sync.dma_start(out=dbg_esb[:, :], in_=esb)
                    return esb

                def emit_mm_c(grp, esb):
                    nq = grp["nq"]
                    for j, (h, sk) in enumerate(grp["chunks"]):
                        key = (nq, h)
                        if key not in pc_tiles:
                            pc_tiles[key] = psC.tile([65, W], f32, tag="pc",
                                                     name=f"pc{nq}_{h}")
                            cnt[key] = 0
                        nc.tensor.matmul(
                            pc_tiles[key],
                            vsb[sk][:, h * 65:h * 65 + 65],
                            esb[:, j * W:(j + 1) * W],
                            start=(cnt[key] == 0), stop=(cnt[key] == SK - 1))
                        cnt[key] += 1
                        if cnt[key] == SK:
                            normalize(pc_tiles[key], nq, h)

                # up-front: QK block 0 (warmup on m=0) + V block 0
                emit_qk_piece(0, 0, warm=True)
                emit_qk_piece(0, 1)
                emit_qk_piece(0, 2)
                emit_v_piece(0)
                emit_v_piece(2)

                HS = 3
                pending = []
                for gi, grp in enumerate(groups):
                    esb = emit_mm_s(gi, grp)
                    pending.append((grp, esb))
                    for fn in pieces.get(gi, []):
                        fn()
                    if gi >= HS:
                        emit_mm_c(*pending.pop(0))
                while pending:
                    emit_mm_c(*pending.pop(0))

                if DEBUG_DUMP:
                    for m in range(3):
                        nc.sync.dma_start(
                            out=dbg_qkt[m * P:(m + 1) * P, :], in_=qkt[m])
                    nc.sync.dma_start(out=dbg_v[:, :], in_=vsb[0])
                    for nq_ in range(NQ):
                        nc.sync.dma_start(
                            out=dbg_ccin[nq_ * 192:(nq_ + 1) * 192, :],
                            in_=cc_in[nq_])

            # ---- gather + output projection ------------------------------
            ctxg = []
            for k in range(KD):
                t = gat.tile([P, W], bf16, tag=f"ctxg{k}", name=f"ctxg{k}")
                nc.gpsimd.indirect_dma_start(
                    out=t,
                    out_offset=None,
                    in_=cc_all[:, :],
                    in_offset=bass.IndirectOffsetOnAxis(ap=gix[k][:, :1],
                                                        axis=0),
                )
                ctxg.append(t)
            if DEBUG_DUMP:
                nc.sync.dma_start(out=dbg_ccall[:, :], in_=cc_all[:, :])
                for k in range(KD):
                    nc.sync.dma_start(
                        out=dbg_ctxg[k * P:(k + 1) * P, :], in_=ctxg[k])
            with tc.tile_pool(name="ps_y", bufs=2, space="PSUM") as py:
                for m in range(KD):
                    ps = py.tile([P, W], f32)
                    if m == 0:
                        # warm the PE during the AllGather wait so the
                        # output projection runs at full clock
                        for d in range(16):
                            nc.tensor.matmul(
                                ps, zeros_t, wo[0][:, 0:W],
                                start=(d == 0), stop=False,
                                skip_group_check=True)
                    for k in range(KD):
                        nc.tensor.matmul(
                            ps,
                            wo[k][:, m * P:(m + 1) * P],
                            ctxg[k],
                            start=(k == 0 and m != 0),
                            stop=(k == KD - 1),
                            skip_group_check=(m == 0))
                    yt = outp.tile([P, W], f32, tag="yt")
                    nc.vector.tensor_scalar_add(yt, ps, bo[m])
                    nc.gpsimd.dma_start(out=out[m * P:(m + 1) * P, :], in_=yt)

    nc.compile()
    return nc


def _get_nc():
    if "nc" not in _CACHE:
        _install_profile_shim()
        _CACHE["nc"] = _build()
    return _CACHE["nc"]


def _make_in_maps(x, Wq, bq, Wk, bk, Wv, bv, Wo, bo):
    import ml_dtypes

    bf16 = ml_dtypes.bfloat16
    scale = np.float32(1.0 / np.sqrt(HD))
    f = np.float32
    x, Wq, bq, Wk, bk, Wv, bv, Wo, bo = [
        np.asarray(a, dtype=f) for a in (x, Wq, bq, Wk, bk, Wv, bv, Wo, bo)]

    in_maps = []
    for c in range(N_CORES):
        b = c // 4
        hs = (c % 4) * HPC
        q = c % 4
        hh = [hs, hs + 1, hs + 2]

        def wc(Wm, h):
            return Wm[:, h * HD:(h + 1) * HD]

        def bc(bm, h):
            return bm[h * HD:(h + 1) * HD]

        xTb = np.ascontiguousarray(x[b].T.astype(bf16))
        w_qk = np.concatenate(
            [wc(Wk, hh[0]), wc(Wk, hh[1]),
             wc(Wq, hh[0]) * scale, wc(Wq, hh[1]) * scale,
             wc(Wk, hh[2]), wc(Wq, hh[2]) * scale], axis=1)
        b_qk = np.concatenate(
            [bc(bk, hh[0]), bc(bk, hh[1]),
             bc(bq, hh[0]) * scale, bc(bq, hh[1]) * scale,
             bc(bk, hh[2]), bc(bq, hh[2]) * scale])[:, None]
        w_v = np.zeros((D, VW), dtype=f)
        b_v = np.zeros((1, VW), dtype=f)
        for i, h in enumerate(hh):
            w_v[:, i * 65:i * 65 + HD] = wc(Wv, h)
            b_v[0, i * 65:i * 65 + HD] = bc(bv, h)
            b_v[0, i * 65 + HD] = 1.0
        g = q * 768 + np.arange(D, dtype=np.uint32)
        in_maps.append({
            "xT": xTb,
            "w_qk": np.ascontiguousarray(w_qk.astype(bf16)),
            "b_qk": np.ascontiguousarray(b_qk),
            "w_v": np.ascontiguousarray(w_v.astype(bf16)),
            "b_v": b_v,
            "w_o": np.ascontiguousarray(Wo.astype(bf16)),
            "b_o": np.ascontiguousarray(bo[:, None]),
            "gidx": g.astype(np.uint32)[:, None],
            "zin": np.zeros((P, P), dtype=bf16),
        })
    return in_maps


def kernel(x, Wq, bq, Wk, bk, Wv, bv, Wo, bo, _trace=False):
    from concourse.bass_utils import run_bass_kernel_spmd

    nc = _get_nc()
    in_maps = _make_in_maps(x, Wq, bq, Wk, bk, Wv, bv, Wo, bo)
    res = run_bass_kernel_spmd(nc, in_maps, list(range(N_CORES)),
                               trace=_trace)
    _CACHE["last_results"] = res
    y = np.empty((B, S, D), dtype=np.float32)
    for c in range(N_CORES):
        b = c // 4
        q = c % 4
        y[b, q * W:(q + 1) * W, :] = res.results[c]["out"].T
    return y


# revision 31
# speedup vs baseline: 1.1549x; 1.0456x over previous
"""Multi-head attention (B=2, S=2048, D=768, H=12) on 8 Trainium2 NeuronCores.

Sharding: core c handles batch b=c//4 and heads 3*(c%4) .. 3*(c%4)+2.

v2: ACT(exp)-centric schedule. The softmax exp is 12.58M elements/core on the
Scalar engine (~0.83ns/col + ~190ns/instr) ~= 100us — the hard floor. The
kernel is organized as one long ACT-saturated attention phase:

  1. All matmul operands are bf16 (fp32 PSUM accumulation): halves x DMA,
     SBUF footprint, and collective bytes. PE rate is unchanged (1 cyc/row
     for both fp32r and bf16).
  2. QK/V projections are split into small pieces and emitted between
     attention groups so the exp stream starts at ~8us; only QK block 0 and
     V block 0 are emitted up front.
  3. Per (s_q 512-quarter): scores^T = K @ Q^T per head (heads paired into PE
     row-halves via tile_position), exp on ScalarE ([128,1024] per
     instruction), ctx^T_aug = V_aug^T @ exp(scores^T) accumulated per head
     in one PSUM bank (ones-column gives the softmax denominator).
  4. Normalize: reciprocal_approx_fast (single DVE op) on the denominator
     row, gpsimd partition_broadcast, one DVE multiply -> bf16 ctx.
  5. Per quarter, one 4-rank batch-group AllGather (bf16) delivers
     ctx^T[768, 512] in head order; core q=c%4 indirect-gathers quarter q and
     computes y^T[:, q*512:(q+1)*512] = Wo^T @ ctx^T + bo with the PE kept
     warm through the collective wait.

Host assembles y[b, q*512:(q+1)*512, :] = out_c^T.
"""
import sys

if "/opt/trn_rl_repo" not in sys.path:
    sys.path.insert(0, "/opt/trn_rl_repo")

import numpy as np

B, S, D, H = 2, 2048, 768, 12
HD = 64
P = 128
N_CORES = 8
HPC = 3          # heads per core
NQ = 4           # s_q chunks of 512
SK = 16          # s_k chunks of 128
KD = 6           # D chunks of 128
W = 512          # working free-dim chunk
VW = 3 * 65      # packed V_aug width (3 heads x (64 + ones column))

_CACHE = {}


def _install_profile_shim():
    """run_bass_kernel_spmd(trace=True) needs antenv.axon_hooks; provide it."""
    import contextlib
    import ctypes
    import types

    if "antenv.axon_hooks" in sys.modules:
        return
    try:
        lib = ctypes.CDLL("/opt/axon/libaxon_pjrt.so")
    except OSError:
        return
    if not hasattr(lib, "axon_start_nrt_profile"):
        return
    lib.axon_start_nrt_profile.argtypes = [
        ctypes.POINTER(ctypes.c_int64),
        ctypes.c_size_t,
    ]
    lib.axon_start_nrt_profile.restype = ctypes.c_int64
    lib.axon_stop_nrt_profile.argtypes = [ctypes.c_char_p]
    lib.axon_stop_nrt_profile.restype = ctypes.c_int64

    @contextlib.contextmanager
    def _hook(output_dir, device_ids):
        import jax

        jax.devices()
        if device_ids:
            ids = (ctypes.c_int64 * len(device_ids))(*device_ids)
            rc = lib.axon_start_nrt_profile(ids, len(device_ids))
        else:
            rc = lib.axon_start_nrt_profile(None, 0)
        if rc != 0:
            raise RuntimeError(f"axon_start_nrt_profile rc={rc}")
        try:
            yield
        finally:
            n = lib.axon_stop_nrt_profile(str(output_dir).encode())
            if n < 0:
                raise RuntimeError(f"axon_stop_nrt_profile rc={n}")

    mod = types.ModuleType("antenv.axon_hooks")
    mod.get_axon_ntff_profile_hook = lambda: _hook
    mod.set_axon_ntff_profile_hook = lambda h: None
    sys.modules["antenv.axon_hooks"] = mod


import os

DEBUG_DUMP = bool(os.environ.get("KERNEL_DEBUG_DUMP"))


def _build():
    import concourse.bass as bass
    from concourse import bacc
    import concourse.tile as tile
    import concourse.mybir as mybir

    bf16 = mybir.dt.bfloat16
    f32 = mybir.dt.float32
    u32 = mybir.dt.uint32
    AF = mybir.ActivationFunctionType
    ALU = mybir.AluOpType

    nc = bacc.Bacc("TRN2", target_bir_lowering=False, debug=False,
                   num_devices=N_CORES)

    xT = nc.dram_tensor("xT", [D, S], bf16, kind="ExternalInput")
    w_qk = nc.dram_tensor("w_qk", [D, 384], bf16, kind="ExternalInput")
    b_qk = nc.dram_tensor("b_qk", [384, 1], f32, kind="ExternalInput")
    w_v = nc.dram_tensor("w_v", [D, VW], bf16, kind="ExternalInput")
    b_v = nc.dram_tensor("b_v", [1, VW], f32, kind="ExternalInput")
    w_o = nc.dram_tensor("w_o", [D, D], bf16, kind="ExternalInput")
    b_o = nc.dram_tensor("b_o", [D, 1], f32, kind="ExternalInput")
    gidx = nc.dram_tensor("gidx", [D, 1], u32, kind="ExternalInput")
    zin = nc.dram_tensor("zin", [P, P], bf16, kind="ExternalInput")
    out = nc.dram_tensor("out", [D, W], f32, kind="ExternalOutput")

    cc_in = nc.dram_tensor("cc_in", [NQ, HPC * HD, W], bf16)
    # 8-rank Shared-output AllGather: 4-rank/Local-output collectives run at
    # ~20GB/s on NRT (staged), 8-rank Shared runs at ~140GB/s.
    cc_all = nc.dram_tensor("cc_all", [NQ * N_CORES * HPC * HD, W], bf16,
                            addr_space="Shared")
    if DEBUG_DUMP:
        dbg_qkt = nc.dram_tensor("dbg_qkt", [3 * P, S], bf16,
                                 kind="ExternalOutput")
        dbg_v = nc.dram_tensor("dbg_v", [P, VW], bf16, kind="ExternalOutput")
        dbg_esb = nc.dram_tensor("dbg_esb", [P, 2 * W], bf16,
                                 kind="ExternalOutput")
        dbg_ccin = nc.dram_tensor("dbg_ccin", [NQ * HPC * HD, W], bf16,
                                  kind="ExternalOutput")
        dbg_ccall = nc.dram_tensor("dbg_ccall", [NQ * N_CORES * HPC * HD, W],
                                   bf16, kind="ExternalOutput")
        dbg_ctxg = nc.dram_tensor("dbg_ctxg", [D, W], bf16,
                                  kind="ExternalOutput")
        dbg_den = nc.dram_tensor("dbg_den", [2, W], f32,
                                 kind="ExternalOutput")
    GROUPS = [list(range(N_CORES))]

    with tile.TileContext(nc) as tc:
        with tc.tile_pool(name="const", bufs=1) as const, \
             tc.tile_pool(name="qkp", bufs=1) as qkp, \
             tc.tile_pool(name="vp", bufs=1) as vp, \
             tc.tile_pool(name="work", bufs=4) as work, \
             tc.tile_pool(name="expp", bufs=6) as expp, \
             tc.tile_pool(name="gat", bufs=1) as gat, \
             tc.tile_pool(name="outp", bufs=3) as outp:

            # ---- constant / input loads -----------------------------------
            # sync queue: zeros then all of x (no deps -> streams in ASAP),
            # then wo/gidx/bo which are only needed at the tail.
            zeros_t = const.tile([P, P], bf16, tag="zeros")
            nc.sync.dma_start(out=zeros_t, in_=zin[:, :])
            xt = [const.tile([P, S], bf16, tag=f"xt{k}", name=f"xt{k}")
                  for k in range(KD)]
            for n in range(NQ):
                for k in range(KD):
                    nc.sync.dma_start(
                        out=xt[k][:, n * W:(n + 1) * W],
                        in_=xT[k * P:(k + 1) * P, n * W:(n + 1) * W])

            # gpsimd queue: projection weights + biases, ordered by deadline
            # (wqk for the warmup/QK block 0, bqk for its bias add, wv/bv for
            # V block 0, then tail-only constants).
            wqk = []
            for k in range(KD):
                t = const.tile([P, 384], bf16, tag=f"wqk{k}")
                nc.gpsimd.dma_start(out=t, in_=w_qk[k * P:(k + 1) * P, :])
                wqk.append(t)
            bqk = []
            for m in range(3):
                t = const.tile([P, 1], f32, tag=f"bqk{m}")
                nc.gpsimd.dma_start(out=t, in_=b_qk[m * P:(m + 1) * P, :])
                bqk.append(t)
            wv = []
            for k in range(KD):
                t = const.tile([P, VW], bf16, tag=f"wv{k}")
                nc.gpsimd.dma_start(out=t, in_=w_v[k * P:(k + 1) * P, :])
                wv.append(t)
            bv = const.tile([P, VW], f32, tag="bv")
            bv_bcast = bass.AP(tensor=b_v[:, :].tensor, offset=0,
                               ap=[[0, P], [1, VW]])
            nc.gpsimd.dma_start(out=bv, in_=bv_bcast)

            # tail constants all on sync: queued behind x, land ~30us, and
            # keep the gpsimd queue (q2c copies, broadcasts, cc writes) clean
            wo = []
            bo = []
            gix = []
            for k in range(KD):
                t = const.tile([P, D], bf16, tag=f"wo{k}")
                nc.sync.dma_start(out=t, in_=w_o[k * P:(k + 1) * P, :])
                wo.append(t)
            for k in range(KD):
                t = const.tile([P, 1], f32, tag=f"bo{k}")
                nc.sync.dma_start(out=t, in_=b_o[k * P:(k + 1) * P, :])
                bo.append(t)
                t = const.tile([P, 1], u32, tag=f"gix{k}")
                nc.sync.dma_start(out=t, in_=gidx[k * P:(k + 1) * P, :])
                gix.append(t)

            # ---- attention state ------------------------------------------
            qkt = [qkp.tile([P, S], bf16, tag=f"qkt{m}", name=f"qkt{m}")
                   for m in range(3)]
            q2c = qkp.tile([64, S], bf16, tag="q2c")
            vsb = [vp.tile([P, VW], bf16, tag=f"v{s}", name=f"v{s}")
                   for s in range(SK)]

            # PSUM budget (8 banks): proj 1 + exp 2x2 + ctx-accum 3
            with tc.tile_pool(name="ps_proj", bufs=1, space="PSUM") as psP, \
                 tc.tile_pool(name="ps_e", bufs=1, space="PSUM") as psE, \
                 tc.tile_pool(name="ps_c", bufs=1, space="PSUM") as psC:

                # ---- projection pieces ------------------------------------
                def emit_qk_piece(n, m, warm=False):
                    ps = psP.tile([P, W], f32, tag="proj",
                                  name=f"psqk{n}_{m}")
                    if warm:
                        # zero-contribution warmup: ramps the PE clock while
                        # the x DMAs land; covers [0:512] so has_written is
                        # clean for the real accumulation below
                        for d in range(16):
                            if d % 2 == 0:
                                nc.tensor.matmul(
                                    ps[:, 0:384], zeros_t, wqk[0][:, :],
                                    start=(d == 0), stop=False,
                                    skip_group_check=True)
                            else:
                                nc.tensor.matmul(
                                    ps[:, 384:512], zeros_t,
                                    wqk[1][:, 0:128],
                                    start=(d == 1), stop=False,
                                    skip_group_check=True)
                    for k in range(KD):
                        nc.tensor.matmul(
                            ps,
                            wqk[k][:, m * P:(m + 1) * P],
                            xt[k][:, n * W:(n + 1) * W],
                            start=(k == 0 and not warm),
                            stop=(k == KD - 1),
                            skip_group_check=warm)
                    nc.vector.tensor_scalar_add(
                        qkt[m][:, n * W:(n + 1) * W], ps, bqk[m])
                    if m == 2:
                        nc.gpsimd.dma_start(
                            out=q2c[:, n * W:(n + 1) * W],
                            in_=qkt[2][64:128, n * W:(n + 1) * W])

                def emit_v_piece(s0):
                    # two V sub-blocks side-by-side in one PSUM tile so the
                    # 12 matmuls run back-to-back (no bias-add wait between)
                    ps = psP.tile([P, W], f32, tag="proj", name=f"psv{s0}")
                    for idx in range(2):
                        s_ = s0 + idx
                        off = idx * 256
                        for k in range(KD):
                            nc.tensor.matmul(
                                ps[:, off:off + VW],
                                xt[k][:, s_ * P:(s_ + 1) * P],
                                wv[k],
                                start=(k == 0), stop=(k == KD - 1))
                    for idx in range(2):
                        s_ = s0 + idx
                        off = idx * 256
                        nc.vector.tensor_tensor(out=vsb[s_],
                                                in0=ps[:, off:off + VW],
                                                in1=bv, op=ALU.add)

                # ---- attention groups -------------------------------------
                # Chunk = one [s_k 128, s_q 512] score block for one head.
                # Groups of 2 chunks share a 2-bank PSUM tile so one ACT exp
                # covers 1024 columns. Software-pipelined emission with ctx
                # lagging scores by 3 groups.
                # Per quarter: pairs (h0,h1) over all sk, then h2 solos.
                # With the pc01/pc2 tag split below, each accumulator is
                # freed ~9-16 groups before its PSUM buffer is reused.
                groups = []
                for nq in range(NQ):
                    for sk in range(SK):
                        groups.append({"nq": nq, "chunks": [(0, sk), (1, sk)]})
                    for sk in range(0, SK, 2):
                        groups.append({"nq": nq,
                                       "chunks": [(2, sk), (2, sk + 1)]})

                # projection pieces interleaved into the group stream:
                # gi -> thunks. K rows for nq0 pairs sk=4n need qk(n,m=0)
                # before gi=4n; vsb[sk] needed by ctx(nq0, sk) at gi=sk+3;
                # K2/Q2 (m=2) by the nq0 solo phase at gi=16; Q block n
                # (m=1) by nq=n's first group (gi=24n).
                pieces = {
                    0: [lambda: emit_v_piece(0)],
                    1: [lambda: emit_qk_piece(0, 2), lambda: emit_v_piece(2)],
                    2: [lambda: emit_qk_piece(1, 0)],
                    3: [lambda: emit_v_piece(4)],
                    4: [lambda: emit_v_piece(6)],
                    5: [lambda: emit_qk_piece(2, 0)],
                    6: [lambda: emit_v_piece(8)],
                    7: [lambda: emit_v_piece(10)],
                    8: [lambda: emit_qk_piece(3, 0)],
                    9: [lambda: emit_v_piece(12)],
                    10: [lambda: emit_v_piece(14)],
                    11: [lambda: emit_qk_piece(1, 2)],
                    13: [lambda: emit_qk_piece(2, 2)],
                    15: [lambda: emit_qk_piece(3, 2)],
                    18: [lambda: emit_qk_piece(1, 1)],
                    27: [lambda: emit_qk_piece(2, 1)],
                    51: [lambda: emit_qk_piece(3, 1)],
                }

                pc_tiles = {}
                cnt = {}
                norm_done = {}

                def normalize(pc, nq, h):
                    # custom-DVE ops drop the input partition base offset, so
                    # stage the denominator row at partition 0 first
                    den = work.tile([1, W], f32, tag="den")
                    nc.vector.tensor_scalar_mul(den, pc[64:65, :], 1.0)
                    rec = work.tile([1, W], f32, tag="rec")
                    nc.vector.reciprocal_approx_fast(rec[0:1, :],
                                                     den[0:1, :])
                    if DEBUG_DUMP and nq == 0 and h == 0:
                        nc.sync.dma_start(out=dbg_den[0:1, :],
                                          in_=den[0:1, :])
                        nc.sync.dma_start(out=dbg_den[1:2, :],
                                          in_=rec[0:1, :])
                    rb = work.tile([64, W], f32, tag="rb")
                    nc.gpsimd.partition_broadcast(rb, rec[:1, :])
                    ctx = work.tile([64, W], bf16, tag="ctx")
                    nc.vector.tensor_tensor(out=ctx, in0=pc[0:64, :], in1=rb,
                                            op=ALU.mult)
                    nc.gpsimd.dma_start(
                        out=cc_in[nq, h * HD:(h + 1) * HD, :],
                        in_=ctx)
                    norm_done.setdefault(nq, set()).add(h)
                    if norm_done[nq] == {0, 1, 2}:
                        nc.gpsimd.collective_compute(
                            "AllGather", ALU.bypass,
                            ins=[cc_in[nq]],
                            outs=[cc_all[nq * 1536:(nq + 1) * 1536, :]],
                            replica_groups=GROUPS)

                def emit_mm_s(gi, grp):
                    nq = grp["nq"]
                    eps = psE.tile([P, 2 * W], f32, tag="ea" if gi % 2 == 0
                                   else "eb", name=f"eps{gi}")
                    for j, (h, sk) in enumerate(grp["chunks"]):
                        if h == 0:
                            lhsT = qkt[0][0:64, sk * P:(sk + 1) * P]
                            rhs = qkt[1][0:64, nq * W:(nq + 1) * W]
                            tp = (0, 0)
                        elif h == 1:
                            lhsT = qkt[0][64:128, sk * P:(sk + 1) * P]
                            rhs = qkt[1][64:128, nq * W:(nq + 1) * W]
                            tp = (64, 0)
                        else:
                            lhsT = qkt[2][0:64, sk * P:(sk + 1) * P]
                            rhs = q2c[:, nq * W:(nq + 1) * W]
                            tp = (0, 0)
                        nc.tensor.matmul(eps[:, j * W:(j + 1) * W], lhsT, rhs,
                                         start=True, stop=True,
                                         tile_position=tp)
                    esb = expp.tile([P, 2 * W], bf16, tag="e",
                                    name=f"esb{gi}")
                    nc.scalar.activation(esb, eps, AF.Exp)
                    if DEBUG_DUMP and gi == 0:
                        nc.sync.dma_start(out=dbg_esb[:, :], in_=esb)
                    return esb

                def emit_mm_c(grp, esb):
                    nq = grp["nq"]
                    for j, (h, sk) in enumerate(grp["chunks"]):
                        key = (nq, h)
                        if key not in pc_tiles:
                            pc_tiles[key] = psC.tile(
                                [65, W], f32,
                                tag="pc2" if h == 2 else "pc01",
                                bufs=1 if h == 2 else 2,
                                name=f"pc{nq}_{h}")
                            cnt[key] = 0
                        nc.tensor.matmul(
                            pc_tiles[key],
                            vsb[sk][:, h * 65:h * 65 + 65],
                            esb[:, j * W:(j + 1) * W],
                            start=(cnt[key] == 0), stop=(cnt[key] == SK - 1))
                        cnt[key] += 1
                        if cnt[key] == SK:
                            normalize(pc_tiles[key], nq, h)

                # up-front: only what group 0 needs (K and Q of block 0);
                # everything else is interleaved via `pieces`
                emit_qk_piece(0, 0, warm=True)
                emit_qk_piece(0, 1)

                HS = 3
                pending = []
                for gi, grp in enumerate(groups):
                    esb = emit_mm_s(gi, grp)
                    pending.append((grp, esb))
                    for fn in pieces.get(gi, []):
                        fn()
                    if gi >= HS:
                        emit_mm_c(*pending.pop(0))
                while pending:
                    emit_mm_c(*pending.pop(0))

                if DEBUG_DUMP:
                    for m in range(3):
                        nc.sync.dma_start(
                            out=dbg_qkt[m * P:(m + 1) * P, :], in_=qkt[m])
                    nc.sync.dma_start(out=dbg_v[:, :], in_=vsb[0])
                    for nq_ in range(NQ):
                        nc.sync.dma_start(
                            out=dbg_ccin[nq_ * 192:(nq_ + 1) * 192, :],
                            in_=cc_in[nq_])

            # ---- gather + output projection ------------------------------
            ctxg = []
            for k in range(KD):
                t = gat.tile([P, W], bf16, tag=f"ctxg{k}", name=f"ctxg{k}")
                nc.gpsimd.indirect_dma_start(
                    out=t,
                    out_offset=None,
                    in_=cc_all[:, :],
                    in_offset=bass.IndirectOffsetOnAxis(ap=gix[k][:, :1],
                                                        axis=0),
                )
                ctxg.append(t)
            if DEBUG_DUMP:
                nc.sync.dma_start(out=dbg_ccall[:, :], in_=cc_all[:, :])
                for k in range(KD):
                    nc.sync.dma_start(
                        out=dbg_ctxg[k * P:(k + 1) * P, :], in_=ctxg[k])
            with tc.tile_pool(name="ps_y", bufs=2, space="PSUM") as py:
                for m in range(KD):
                    ps = py.tile([P, W], f32)
                    if m == 0:
                        # warm the PE during the AllGather wait so the
                        # output projection runs at full clock
                        for d in range(40):
                            nc.tensor.matmul(
                                ps, zeros_t, wo[0][:, 0:W],
                                start=(d == 0), stop=False,
                                skip_group_check=True)
                    for k in range(KD):
                        nc.tensor.matmul(
                            ps,
                            wo[k][:, m * P:(m + 1) * P],
                            ctxg[k],
                            start=(k == 0 and m != 0),
                            stop=(k == KD - 1),
                            skip_group_check=(m == 0))
                    yt = outp.tile([P, W], f32, tag="yt")
                    nc.vector.tensor_scalar_add(yt, ps, bo[m])
                    nc.gpsimd.dma_start(out=out[m * P:(m + 1) * P, :], in_=yt)

    nc.compile()
    return nc


def _get_nc():
    if "nc" not in _CACHE:
        _install_profile_shim()
        _CACHE["nc"] = _build()
    return _CACHE["nc"]


def _make_in_maps(x, Wq, bq, Wk, bk, Wv, bv, Wo, bo):
    import ml_dtypes

    bf16 = ml_dtypes.bfloat16
    scale = np.float32(1.0 / np.sqrt(HD))
    f = np.float32
    x, Wq, bq, Wk, bk, Wv, bv, Wo, bo = [
        np.asarray(a, dtype=f) for a in (x, Wq, bq, Wk, bk, Wv, bv, Wo, bo)]

    in_maps = []
    for c in range(N_CORES):
        b = c // 4
        hs = (c % 4) * HPC
        q = c % 4
        hh = [hs, hs + 1, hs + 2]

        def wc(Wm, h):
            return Wm[:, h * HD:(h + 1) * HD]

        def bc(bm, h):
            return bm[h * HD:(h + 1) * HD]

        xTb = np.ascontiguousarray(x[b].T.astype(bf16))
        w_qk = np.concatenate(
            [wc(Wk, hh[0]), wc(Wk, hh[1]),
             wc(Wq, hh[0]) * scale, wc(Wq, hh[1]) * scale,
             wc(Wk, hh[2]), wc(Wq, hh[2]) * scale], axis=1)
        b_qk = np.concatenate(
            [bc(bk, hh[0]), bc(bk, hh[1]),
             bc(bq, hh[0]) * scale, bc(bq, hh[1]) * scale,
             bc(bk, hh[2]), bc(bq, hh[2]) * scale])[:, None]
        w_v = np.zeros((D, VW), dtype=f)
        b_v = np.zeros((1, VW), dtype=f)
        for i, h in enumerate(hh):
            w_v[:, i * 65:i * 65 + HD] = wc(Wv, h)
            b_v[0, i * 65:i * 65 + HD] = bc(bv, h)
            b_v[0, i * 65 + HD] = 1.0
        i_feat = np.arange(D, dtype=np.uint32)
        g = q * 1536 + (4 * b + i_feat // 192) * 192 + (i_feat % 192)
        in_maps.append({
            "xT": xTb,
            "w_qk": np.ascontiguousarray(w_qk.astype(bf16)),
            "b_qk": np.ascontiguousarray(b_qk),
            "w_v": np.ascontiguousarray(w_v.astype(bf16)),
            "b_v": b_v,
            "w_o": np.ascontiguousarray(Wo.astype(bf16)),
            "b_o": np.ascontiguousarray(bo[:, None]),
            "gidx": g.astype(np.uint32)[:, None],
            "zin": np.zeros((P, P), dtype=bf16),
        })
    return in_maps


def kernel(x, Wq, bq, Wk, bk, Wv, bv, Wo, bo, _trace=False):
    from concourse.bass_utils import run_bass_kernel_spmd

    nc = _get_nc()
    in_maps = _make_in_maps(x, Wq, bq, Wk, bk, Wv, bv, Wo, bo)
    res = run_bass_kernel_spmd(nc, in_maps, list(range(N_CORES)),
                               trace=_trace)
    _CACHE["last_results"] = res
    y = np.empty((B, S, D), dtype=np.float32)
    for c in range(N_CORES):
        b = c // 4
        q = c % 4
        y[b, q * W:(q + 1) * W, :] = res.results[c]["out"].T
    return y


# revision 35
# speedup vs baseline: 1.2010x; 1.0399x over previous
"""Multi-head attention (B=2, S=2048, D=768, H=12) on 8 Trainium2 NeuronCores.

Sharding: core c handles batch b=c//4 and heads 3*(c%4) .. 3*(c%4)+2.

v2: ACT(exp)-centric schedule. The softmax exp is 12.58M elements/core on the
Scalar engine (~0.83ns/col + ~190ns/instr) ~= 100us — the hard floor. The
kernel is organized as one long ACT-saturated attention phase:

  1. All matmul operands are bf16 (fp32 PSUM accumulation): halves x DMA,
     SBUF footprint, and collective bytes. PE rate is unchanged (1 cyc/row
     for both fp32r and bf16).
  2. QK/V projections are split into small pieces and emitted between
     attention groups so the exp stream starts at ~8us; only QK block 0 and
     V block 0 are emitted up front.
  3. Per (s_q 512-quarter): scores^T = K @ Q^T per head (heads paired into PE
     row-halves via tile_position), exp on ScalarE ([128,1024] per
     instruction), ctx^T_aug = V_aug^T @ exp(scores^T) accumulated per head
     in one PSUM bank (ones-column gives the softmax denominator).
  4. Normalize: reciprocal_approx_fast (single DVE op) on the denominator
     row, gpsimd partition_broadcast, one DVE multiply -> bf16 ctx.
  5. Per quarter, one 4-rank batch-group AllGather (bf16) delivers
     ctx^T[768, 512] in head order; core q=c%4 indirect-gathers quarter q and
     computes y^T[:, q*512:(q+1)*512] = Wo^T @ ctx^T + bo with the PE kept
     warm through the collective wait.

Host assembles y[b, q*512:(q+1)*512, :] = out_c^T.
"""
import sys

if "/opt/trn_rl_repo" not in sys.path:
    sys.path.insert(0, "/opt/trn_rl_repo")

import numpy as np

B, S, D, H = 2, 2048, 768, 12
HD = 64
P = 128
N_CORES = 8
HPC = 3          # heads per core
NQ = 4           # s_q chunks of 512
SK = 16          # s_k chunks of 128
KD = 6           # D chunks of 128
W = 512          # working free-dim chunk
VW = 3 * 65      # packed V_aug width (3 heads x (64 + ones column))

_CACHE = {}


def _install_profile_shim():
    """run_bass_kernel_spmd(trace=True) needs antenv.axon_hooks; provide it."""
    import contextlib
    import ctypes
    import types

    if "antenv.axon_hooks" in sys.modules:
        return
    try:
        lib = ctypes.CDLL("/opt/axon/libaxon_pjrt.so")
    except OSError:
        return
    if not hasattr(lib, "axon_start_nrt_profile"):
        return
    lib.axon_start_nrt_profile.argtypes = [
        ctypes.POINTER(ctypes.c_int64),
        ctypes.c_size_t,
    ]
    lib.axon_start_nrt_profile.restype = ctypes.c_int64
    lib.axon_stop_nrt_profile.argtypes = [ctypes.c_char_p]
    lib.axon_stop_nrt_profile.restype = ctypes.c_int64

    @contextlib.contextmanager
    def _hook(output_dir, device_ids):
        import jax

        jax.devices()
        if device_ids:
            ids = (ctypes.c_int64 * len(device_ids))(*device_ids)
            rc = lib.axon_start_nrt_profile(ids, len(device_ids))
        else:
            rc = lib.axon_start_nrt_profile(None, 0)
        if rc != 0:
            raise RuntimeError(f"axon_start_nrt_profile rc={rc}")
        try:
            yield
        finally:
            n = lib.axon_stop_nrt_profile(str(output_dir).encode())
            if n < 0:
                raise RuntimeError(f"axon_stop_nrt_profile rc={n}")

    mod = types.ModuleType("antenv.axon_hooks")
    mod.get_axon_ntff_profile_hook = lambda: _hook
    mod.set_axon_ntff_profile_hook = lambda h: None
    sys.modules["antenv.axon_hooks"] = mod


import os

DEBUG_DUMP = bool(os.environ.get("KERNEL_DEBUG_DUMP"))


def _build():
    import concourse.bass as bass
    from concourse import bacc
    import concourse.tile as tile
    import concourse.mybir as mybir

    bf16 = mybir.dt.bfloat16
    f32 = mybir.dt.float32
    u32 = mybir.dt.uint32
    AF = mybir.ActivationFunctionType
    ALU = mybir.AluOpType

    nc = bacc.Bacc("TRN2", target_bir_lowering=False, debug=False,
                   num_devices=N_CORES)

    xT = nc.dram_tensor("xT", [D, S], bf16, kind="ExternalInput")
    w_qk = nc.dram_tensor("w_qk", [D, 384], bf16, kind="ExternalInput")
    b_qk = nc.dram_tensor("b_qk", [384, 1], f32, kind="ExternalInput")
    w_v = nc.dram_tensor("w_v", [D, VW], bf16, kind="ExternalInput")
    b_v = nc.dram_tensor("b_v", [1, VW], f32, kind="ExternalInput")
    w_o = nc.dram_tensor("w_o", [D, D], bf16, kind="ExternalInput")
    b_o = nc.dram_tensor("b_o", [D, 1], f32, kind="ExternalInput")
    gidx = nc.dram_tensor("gidx", [D, 1], u32, kind="ExternalInput")
    zin = nc.dram_tensor("zin", [P, P], bf16, kind="ExternalInput")
    out = nc.dram_tensor("out", [D, W], f32, kind="ExternalOutput")

    cc_in = nc.dram_tensor("cc_in", [NQ, HPC * HD, W], bf16)
    # 8-rank Shared-output AllGather: 4-rank/Local-output collectives run at
    # ~20GB/s on NRT (staged), 8-rank Shared runs at ~140GB/s.
    cc_all = nc.dram_tensor("cc_all", [NQ * N_CORES * HPC * HD, W], bf16,
                            addr_space="Shared")
    if DEBUG_DUMP:
        dbg_qkt = nc.dram_tensor("dbg_qkt", [3 * P, S], bf16,
                                 kind="ExternalOutput")
        dbg_v = nc.dram_tensor("dbg_v", [P, VW], bf16, kind="ExternalOutput")
        dbg_esb = nc.dram_tensor("dbg_esb", [P, 2 * W], bf16,
                                 kind="ExternalOutput")
        dbg_ccin = nc.dram_tensor("dbg_ccin", [NQ * HPC * HD, W], bf16,
                                  kind="ExternalOutput")
        dbg_ccall = nc.dram_tensor("dbg_ccall", [NQ * N_CORES * HPC * HD, W],
                                   bf16, kind="ExternalOutput")
        dbg_ctxg = nc.dram_tensor("dbg_ctxg", [D, W], bf16,
                                  kind="ExternalOutput")
        dbg_den = nc.dram_tensor("dbg_den", [2, W], f32,
                                 kind="ExternalOutput")
    GROUPS = [list(range(N_CORES))]

    with tile.TileContext(nc) as tc:
        with tc.tile_pool(name="const", bufs=1) as const, \
             tc.tile_pool(name="qkp", bufs=1) as qkp, \
             tc.tile_pool(name="vp", bufs=1) as vp, \
             tc.tile_pool(name="work", bufs=4) as work, \
             tc.tile_pool(name="expp", bufs=6) as expp, \
             tc.tile_pool(name="gat", bufs=1) as gat, \
             tc.tile_pool(name="outp", bufs=3) as outp:

            # ---- constant / input loads -----------------------------------
            # sync queue: zeros then all of x (no deps -> streams in ASAP),
            # then wo/gidx/bo which are only needed at the tail.
            zeros_t = const.tile([P, P], bf16, tag="zeros")
            nc.sync.dma_start(out=zeros_t, in_=zin[:, :])
            xt = [const.tile([P, S], bf16, tag=f"xt{k}", name=f"xt{k}")
                  for k in range(KD)]
            for n in range(NQ):
                for k in range(KD):
                    nc.sync.dma_start(
                        out=xt[k][:, n * W:(n + 1) * W],
                        in_=xT[k * P:(k + 1) * P, n * W:(n + 1) * W])

            # gpsimd queue: projection weights + biases, ordered by deadline
            # (wqk for the warmup/QK block 0, bqk for its bias add, wv/bv for
            # V block 0, then tail-only constants).
            wqk = []
            for k in range(KD):
                t = const.tile([P, 384], bf16, tag=f"wqk{k}")
                nc.gpsimd.dma_start(out=t, in_=w_qk[k * P:(k + 1) * P, :])
                wqk.append(t)
            bqk = []
            for m in range(3):
                t = const.tile([P, 1], f32, tag=f"bqk{m}")
                nc.gpsimd.dma_start(out=t, in_=b_qk[m * P:(m + 1) * P, :])
                bqk.append(t)
            wv = []
            for k in range(KD):
                t = const.tile([P, VW], bf16, tag=f"wv{k}")
                nc.gpsimd.dma_start(out=t, in_=w_v[k * P:(k + 1) * P, :])
                wv.append(t)
            bv = const.tile([P, VW], f32, tag="bv")
            bv_bcast = bass.AP(tensor=b_v[:, :].tensor, offset=0,
                               ap=[[0, P], [1, VW]])
            nc.gpsimd.dma_start(out=bv, in_=bv_bcast)

            # tail constants all on sync: queued behind x, land ~30us, and
            # keep the gpsimd queue (q2c copies, broadcasts, cc writes) clean
            wo = []
            bo = []
            gix = []
            for k in range(KD):
                t = const.tile([P, D], bf16, tag=f"wo{k}")
                nc.sync.dma_start(out=t, in_=w_o[k * P:(k + 1) * P, :])
                wo.append(t)
            for k in range(KD):
                t = const.tile([P, 1], f32, tag=f"bo{k}")
                nc.sync.dma_start(out=t, in_=b_o[k * P:(k + 1) * P, :])
                bo.append(t)
                t = const.tile([P, 1], u32, tag=f"gix{k}")
                nc.sync.dma_start(out=t, in_=gidx[k * P:(k + 1) * P, :])
                gix.append(t)

            # ---- attention state ------------------------------------------
            qkt = [qkp.tile([P, S], bf16, tag=f"qkt{m}", name=f"qkt{m}")
                   for m in range(3)]
            q2c = qkp.tile([64, S], bf16, tag="q2c")
            vsb = [vp.tile([P, VW], bf16, tag=f"v{s}", name=f"v{s}")
                   for s in range(SK)]

            # PSUM budget (8 banks): proj 1 + exp 2x2 + ctx-accum 3
            with tc.tile_pool(name="ps_proj", bufs=1, space="PSUM") as psP, \
                 tc.tile_pool(name="ps_e", bufs=1, space="PSUM") as psE, \
                 tc.tile_pool(name="ps_c", bufs=1, space="PSUM") as psC:

                # ---- projection pieces ------------------------------------
                def emit_qk_piece(n, m, warm=False, tag="proj", bufs=1):
                    pool = psC if tag == "pc01" else psP
                    ps = pool.tile([P, W], f32, tag=tag, bufs=bufs,
                                   name=f"psqk{n}_{m}")
                    if warm:
                        # zero-contribution warmup: ramps the PE clock while
                        # the x DMAs land; covers [0:512] so has_written is
                        # clean for the real accumulation below
                        for d in range(24):
                            if d % 2 == 0:
                                nc.tensor.matmul(
                                    ps[:, 0:384], zeros_t, wqk[0][:, :],
                                    start=(d == 0), stop=False,
                                    skip_group_check=True)
                            else:
                                nc.tensor.matmul(
                                    ps[:, 384:512], zeros_t,
                                    wqk[1][:, 0:128],
                                    start=(d == 1), stop=False,
                                    skip_group_check=True)
                    for k in range(KD):
                        nc.tensor.matmul(
                            ps,
                            wqk[k][:, m * P:(m + 1) * P],
                            xt[k][:, n * W:(n + 1) * W],
                            start=(k == 0 and not warm),
                            stop=(k == KD - 1),
                            skip_group_check=warm)
                    nc.vector.tensor_scalar_add(
                        qkt[m][:, n * W:(n + 1) * W], ps, bqk[m])
                    if m == 2:
                        nc.gpsimd.dma_start(
                            out=q2c[:, n * W:(n + 1) * W],
                            in_=qkt[2][64:128, n * W:(n + 1) * W])

                def emit_v_piece(s0):
                    # two V sub-blocks side-by-side in one PSUM tile so the
                    # 12 matmuls run back-to-back (no bias-add wait between)
                    ps = psP.tile([P, W], f32, tag="proj", name=f"psv{s0}")
                    for idx in range(2):
                        s_ = s0 + idx
                        off = idx * 256
                        for k in range(KD):
                            nc.tensor.matmul(
                                ps[:, off:off + VW],
                                xt[k][:, s_ * P:(s_ + 1) * P],
                                wv[k],
                                start=(k == 0), stop=(k == KD - 1))
                    for idx in range(2):
                        s_ = s0 + idx
                        off = idx * 256
                        nc.vector.tensor_tensor(out=vsb[s_],
                                                in0=ps[:, off:off + VW],
                                                in1=bv, op=ALU.add)

                # ---- attention groups -------------------------------------
                # Chunk = one [s_k 128, s_q 512] score block for one head.
                # Groups of 2 chunks share a 2-bank PSUM tile so one ACT exp
                # covers 1024 columns. Software-pipelined emission with ctx
                # lagging scores by 3 groups.
                # Per quarter: pairs (h0,h1) over all sk, then h2 solos.
                # With the pc01/pc2 tag split below, each accumulator is
                # freed ~9-16 groups before its PSUM buffer is reused.
                groups = []
                for nq in range(NQ):
                    for sk in range(SK):
                        groups.append({"nq": nq, "chunks": [(0, sk), (1, sk)]})
                    for sk in range(0, SK, 2):
                        groups.append({"nq": nq,
                                       "chunks": [(2, sk), (2, sk + 1)]})

                # projection pieces interleaved into the group stream:
                # gi -> thunks. K rows for nq0 pairs sk=4n need qk(n,m=0)
                # before gi=4n; vsb[sk] needed by ctx(nq0, sk) at gi=sk+3;
                # K2/Q2 (m=2) by the nq0 solo phase at gi=16; Q block n
                # (m=1) by nq=n's first group (gi=24n).
                pieces = {
                    0: [lambda: emit_v_piece(0)],
                    1: [lambda: emit_qk_piece(0, 2), lambda: emit_v_piece(2)],
                    2: [lambda: emit_qk_piece(1, 0)],
                    3: [lambda: emit_v_piece(4)],
                    4: [lambda: emit_v_piece(6)],
                    5: [lambda: emit_qk_piece(2, 0)],
                    6: [lambda: emit_v_piece(8)],
                    7: [lambda: emit_v_piece(10)],
                    8: [lambda: emit_qk_piece(3, 0)],
                    9: [lambda: emit_v_piece(12)],
                    10: [lambda: emit_v_piece(14)],
                    15: [lambda: emit_qk_piece(1, 2)],
                    17: [lambda: emit_qk_piece(2, 2)],
                    19: [lambda: emit_qk_piece(3, 2)],
                    21: [lambda: emit_qk_piece(1, 1)],
                    27: [lambda: emit_qk_piece(2, 1)],
                    51: [lambda: emit_qk_piece(3, 1)],
                }

                pc_tiles = {}
                cnt = {}
                norm_done = {}

                def normalize(pc, nq, h):
                    # custom-DVE ops drop the input partition base offset, so
                    # stage the denominator row at partition 0 first
                    den = work.tile([1, W], f32, tag="den")
                    nc.vector.tensor_scalar_mul(den, pc[64:65, :], 1.0)
                    rec = work.tile([1, W], f32, tag="rec")
                    nc.vector.reciprocal_approx_fast(rec[0:1, :],
                                                     den[0:1, :])
                    if DEBUG_DUMP and nq == 0 and h == 0:
                        nc.sync.dma_start(out=dbg_den[0:1, :],
                                          in_=den[0:1, :])
                        nc.sync.dma_start(out=dbg_den[1:2, :],
                                          in_=rec[0:1, :])
                    rb = work.tile([64, W], f32, tag="rb")
                    nc.gpsimd.partition_broadcast(rb, rec[:1, :])
                    ctx = work.tile([64, W], bf16, tag="ctx")
                    nc.vector.tensor_tensor(out=ctx, in0=pc[0:64, :], in1=rb,
                                            op=ALU.mult)
                    nc.gpsimd.dma_start(
                        out=cc_in[nq, h * HD:(h + 1) * HD, :],
                        in_=ctx)
                    norm_done.setdefault(nq, set()).add(h)
                    if norm_done[nq] == {0, 1, 2}:
                        nc.gpsimd.collective_compute(
                            "AllGather", ALU.bypass,
                            ins=[cc_in[nq]],
                            outs=[cc_all[nq * 1536:(nq + 1) * 1536, :]],
                            replica_groups=GROUPS)

                def emit_mm_s(gi, grp):
                    nq = grp["nq"]
                    eps = psE.tile([P, 2 * W], f32, tag="ea" if gi % 2 == 0
                                   else "eb", name=f"eps{gi}")
                    for j, (h, sk) in enumerate(grp["chunks"]):
                        if h == 0:
                            lhsT = qkt[0][0:64, sk * P:(sk + 1) * P]
                            rhs = qkt[1][0:64, nq * W:(nq + 1) * W]
                            tp = (0, 0)
                        elif h == 1:
                            lhsT = qkt[0][64:128, sk * P:(sk + 1) * P]
                            rhs = qkt[1][64:128, nq * W:(nq + 1) * W]
                            tp = (64, 0)
                        else:
                            lhsT = qkt[2][0:64, sk * P:(sk + 1) * P]
                            rhs = q2c[:, nq * W:(nq + 1) * W]
                            tp = (0, 0)
                        nc.tensor.matmul(eps[:, j * W:(j + 1) * W], lhsT, rhs,
                                         start=True, stop=True,
                                         tile_position=tp)
                    esb = expp.tile([P, 2 * W], bf16, tag="e",
                                    name=f"esb{gi}")
                    nc.scalar.activation(esb, eps, AF.Exp)
                    if DEBUG_DUMP and gi == 0:
                        nc.sync.dma_start(out=dbg_esb[:, :], in_=esb)
                    return esb

                def emit_mm_c(grp, esb):
                    nq = grp["nq"]
                    for j, (h, sk) in enumerate(grp["chunks"]):
                        key = (nq, h)
                        if key not in pc_tiles:
                            pc_tiles[key] = psC.tile(
                                [65, W], f32,
                                tag="pc2" if h == 2 else "pc01",
                                bufs=1 if h == 2 else 2,
                                name=f"pc{nq}_{h}")
                            cnt[key] = 0
                        nc.tensor.matmul(
                            pc_tiles[key],
                            vsb[sk][:, h * 65:h * 65 + 65],
                            esb[:, j * W:(j + 1) * W],
                            start=(cnt[key] == 0), stop=(cnt[key] == SK - 1))
                        cnt[key] += 1
                        if cnt[key] == SK:
                            normalize(pc_tiles[key], nq, h)

                # up-front: only what group 0 needs (K and Q of block 0),
                # placed in the two pc01 PSUM banks so the head stream has
                # no proj-buffer wait (keeps PE duty high -> early HAM
                # boost); everything else is interleaved via `pieces`
                emit_qk_piece(0, 0, warm=True, tag="pc01", bufs=2)
                emit_qk_piece(0, 1, tag="pc01", bufs=2)

                HS = 3
                pending = []
                for gi, grp in enumerate(groups):
                    esb = emit_mm_s(gi, grp)
                    pending.append((grp, esb))
                    for fn in pieces.get(gi, []):
                        fn()
                    if gi >= HS:
                        emit_mm_c(*pending.pop(0))
                while pending:
                    emit_mm_c(*pending.pop(0))

                if DEBUG_DUMP:
                    for m in range(3):
                        nc.sync.dma_start(
                            out=dbg_qkt[m * P:(m + 1) * P, :], in_=qkt[m])
                    nc.sync.dma_start(out=dbg_v[:, :], in_=vsb[0])
                    for nq_ in range(NQ):
                        nc.sync.dma_start(
                            out=dbg_ccin[nq_ * 192:(nq_ + 1) * 192, :],
                            in_=cc_in[nq_])

            # ---- gather + output projection ------------------------------
            ctxg = []
            for k in range(KD):
                t = gat.tile([P, W], bf16, tag=f"ctxg{k}", name=f"ctxg{k}")
                nc.gpsimd.indirect_dma_start(
                    out=t,
                    out_offset=None,
                    in_=cc_all[:, :],
                    in_offset=bass.IndirectOffsetOnAxis(ap=gix[k][:, :1],
                                                        axis=0),
                )
                ctxg.append(t)
            if DEBUG_DUMP:
                nc.sync.dma_start(out=dbg_ccall[:, :], in_=cc_all[:, :])
                for k in range(KD):
                    nc.sync.dma_start(
                        out=dbg_ctxg[k * P:(k + 1) * P, :], in_=ctxg[k])
            with tc.tile_pool(name="ps_y", bufs=2, space="PSUM") as py:
                for m in range(KD):
                    ps = py.tile([P, W], f32)
                    if m == 0:
                        # warm the PE through the AllGather wait (~25us) so
                        # the output projection runs at full clock
                        for d in range(120):
                            nc.tensor.matmul(
                                ps, zeros_t, wo[0][:, 0:W],
                                start=(d == 0), stop=False,
                                skip_group_check=True)
                    for k in range(KD):
                        nc.tensor.matmul(
                            ps,
                            wo[k][:, m * P:(m + 1) * P],
                            ctxg[k],
                            start=(k == 0 and m != 0),
                            stop=(k == KD - 1),
                            skip_group_check=(m == 0))
                    yt = outp.tile([P, W], f32, tag="yt")
                    nc.vector.tensor_scalar_add(yt, ps, bo[m])
                    nc.gpsimd.dma_start(out=out[m * P:(m + 1) * P, :], in_=yt)

    nc.compile()
    return nc


def _get_nc():
    if "nc" not in _CACHE:
        _install_profile_shim()
        _CACHE["nc"] = _build()
    return _CACHE["nc"]


def _make_in_maps(x, Wq, bq, Wk, bk, Wv, bv, Wo, bo):
    import ml_dtypes

    bf16 = ml_dtypes.bfloat16
    scale = np.float32(1.0 / np.sqrt(HD))
    f = np.float32
    x, Wq, bq, Wk, bk, Wv, bv, Wo, bo = [
        np.asarray(a, dtype=f) for a in (x, Wq, bq, Wk, bk, Wv, bv, Wo, bo)]

    in_maps = []
    for c in range(N_CORES):
        b = c // 4
        hs = (c % 4) * HPC
        q = c % 4
        hh = [hs, hs + 1, hs + 2]

        def wc(Wm, h):
            return Wm[:, h * HD:(h + 1) * HD]

        def bc(bm, h):
            return bm[h * HD:(h + 1) * HD]

        xTb = np.ascontiguousarray(x[b].T.astype(bf16))
        w_qk = np.concatenate(
            [wc(Wk, hh[0]), wc(Wk, hh[1]),
             wc(Wq, hh[0]) * scale, wc(Wq, hh[1]) * scale,
             wc(Wk, hh[2]), wc(Wq, hh[2]) * scale], axis=1)
        b_qk = np.concatenate(
            [bc(bk, hh[0]), bc(bk, hh[1]),
             bc(bq, hh[0]) * scale, bc(bq, hh[1]) * scale,
             bc(bk, hh[2]), bc(bq, hh[2]) * scale])[:, None]
        w_v = np.zeros((D, VW), dtype=f)
        b_v = np.zeros((1, VW), dtype=f)
        for i, h in enumerate(hh):
            w_v[:, i * 65:i * 65 + HD] = wc(Wv, h)
            b_v[0, i * 65:i * 65 + HD] = bc(bv, h)
            b_v[0, i * 65 + HD] = 1.0
        i_feat = np.arange(D, dtype=np.uint32)
        g = q * 1536 + (4 * b + i_feat // 192) * 192 + (i_feat % 192)
        in_maps.append({
            "xT": xTb,
            "w_qk": np.ascontiguousarray(w_qk.astype(bf16)),
            "b_qk": np.ascontiguousarray(b_qk),
            "w_v": np.ascontiguousarray(w_v.astype(bf16)),
            "b_v": b_v,
            "w_o": np.ascontiguousarray(Wo.astype(bf16)),
            "b_o": np.ascontiguousarray(bo[:, None]),
            "gidx": g.astype(np.uint32)[:, None],
            "zin": np.zeros((P, P), dtype=bf16),
        })
    return in_maps


def kernel(x, Wq, bq, Wk, bk, Wv, bv, Wo, bo, _trace=False):
    from concourse.bass_utils import run_bass_kernel_spmd

    nc = _get_nc()
    in_maps = _make_in_maps(x, Wq, bq, Wk, bk, Wv, bv, Wo, bo)
    res = run_bass_kernel_spmd(nc, in_maps, list(range(N_CORES)),
                               trace=_trace)
    _CACHE["last_results"] = res
    y = np.empty((B, S, D), dtype=np.float32)
    for c in range(N_CORES):
        b = c // 4
        q = c % 4
        y[b, q * W:(q + 1) * W, :] = res.results[c]["out"].T
    return y
